# revision 1
# baseline (speedup 1.0000x reference)
"""Trainium2 Bass kernel for nn_Dot_Attention (sparse attention softmax).

Computes, for each mention m:
    alpha[m, s] = (queries[m] . values[m, s]) / sqrt(D)
    valid[m, s] = (s < len[m]) & ~(start[m] <= s < end[m])
    out[m, :]   = softmax(where(valid, alpha, -inf))

Sharding: mention dim (axis 0) split evenly across 8 NeuronCores (pure data
parallel). Host does only the tiny index gathers (len/start/end per mention,
derived from idx/lengths) and the final concat; all heavy compute (dot
products, masking, softmax over 1.5 GiB of values) runs on device.

Device strategy (per core: 256 mentions = 2 blocks of 128):
  - values tiles loaded as [128 mentions, SG, 384] (one contiguous 24 KB run
    per partition, 3 MB per DMA) -> mentions on partitions, so alpha lands
    directly in the softmax-friendly [mention, s] layout; no transposes.
  - dot products: VectorE does ONE pass (tensor_mul of PR s-positions
    against a PR-replicated q); ScalarE does the reduction over D via
    activation(Copy, scale=1/sqrt(D), accum_out=alpha column). This keeps
    VectorE at ~1 pass/element (fp32 tensor_tensor is 1x mode) and puts the
    reduce on the otherwise-idle ScalarE. (tensor_tensor_reduce would fuse
    both on DVE but crashes the exec unit on this hardware/runtime.)
  - mask built from an iota row vs per-partition scalars; applied as
    alpha - C*invalid before a single ScalarE Exp that also emits the row
    sums via accum_out; normalize with reciprocal + tensor_scalar (2x mode).
"""

import math

import numpy as np

M, S, D = 2048, 512, 384
NCORES = 8
ML = M // NCORES          # mentions per core
BLK = 128                 # mentions per block (partition dim)
NBLK = ML // BLK
SG = 16                   # s-positions per values DMA tile (3 MB per DMA)
PR = 4                    # s-positions fused per DVE multiply
SCALE = 1.0 / math.sqrt(D)
BIGC = 300.0              # exp(x - BIGC) == 0.0 in fp32 for masked entries

_NC = {}


def _build(rep=1):
    """Build+compile the per-core Bass module. rep>1 unrolls the whole
    computation rep times (used only by test.py for slope-based timing)."""
    if rep in _NC:
        return _NC[rep]

    import concourse.bacc as bacc
    import concourse.tile as tile
    import concourse.mybir as mybir

    F32 = mybir.dt.float32
    Op = mybir.AluOpType

    nc = bacc.Bacc(
        "TRN2", target_bir_lowering=False, debug=False, num_devices=NCORES
    )
    q_ap = nc.dram_tensor("queries", [ML, D], F32, kind="ExternalInput").ap()
    v_ap = nc.dram_tensor("values", [ML, S, D], F32, kind="ExternalInput").ap()
    s3_ap = nc.dram_tensor("scal3", [ML, 3], F32, kind="ExternalInput").ap()
    io_ap = nc.dram_tensor("iota", [BLK, S], F32, kind="ExternalInput").ap()
    out_ap = nc.dram_tensor("out", [ML, S], F32, kind="ExternalOutput").ap()

    with tile.TileContext(nc) as tc:
        with (
            tc.tile_pool(name="pv", bufs=3) as pv,
            tc.tile_pool(name="pq", bufs=2) as pq,
            tc.tile_pool(name="pa", bufs=2) as pa,
            tc.tile_pool(name="ps", bufs=2) as ps,
            tc.tile_pool(name="pc", bufs=1) as pc,
        ):
            iota_t = pc.tile([BLK, S], F32)
            nc.scalar.dma_start(iota_t[:], io_ap)

            for b in [bb for _ in range(rep) for bb in range(NBLK)]:
                m0 = b * BLK
                # q replicated PR times along free dim so one DVE multiply
                # covers PR s-positions (bigger ops amortize DVE overhead)
                q4 = pq.tile([BLK, PR, D], F32, tag="q4")
                for k in range(PR):
                    nc.scalar.dma_start(q4[:, k, :], q_ap[m0 : m0 + BLK, :])
                sc_t = pq.tile([BLK, 3], F32, tag="sc3")
                nc.scalar.dma_start(sc_t[:], s3_ap[m0 : m0 + BLK, :])

                alpha = pa.tile([BLK, S], F32, tag="alpha")
                for g in range(S // SG):
                    v_t = pv.tile([BLK, SG, D], F32, tag="v")
                    # big V loads stay on the SP HWDGE ring only: putting them
                    # on the ACT ring stalls the activation stream (+40% e2e)
                    nc.sync.dma_start(
                        v_t[:], v_ap[m0 : m0 + BLK, g * SG : (g + 1) * SG, :]
                    )
                    for j0 in range(0, SG, PR):
                        # DVE: one-pass elementwise product for PR s-positions
                        prod = ps.tile([BLK, PR, D], F32, tag="prod", bufs=3)
                        nc.vector.tensor_mul(
                            prod[:], v_t[:, j0 : j0 + PR, :], q4[:]
                        )
                        for k in range(PR):
                            s_idx = g * SG + j0 + k
                            # ACT: reduce over D via activation accum_out
                            # (also applies the 1/sqrt(D) scale)
                            dump = ps.tile([BLK, D], F32, tag="dump", bufs=2)
                            nc.scalar.activation(
                                dump[:],
                                prod[:, k, :],
                                mybir.ActivationFunctionType.Copy,
                                bias=0.0,
                                scale=SCALE,
                                accum_out=alpha[:, s_idx : s_idx + 1],
                            )

                # invalid = (iota >= len) | ((iota >= start) & (iota < end))
                mA = ps.tile([BLK, S], F32, tag="mA")
                nc.vector.tensor_scalar(mA[:], iota_t[:], sc_t[:, 0:1], None, Op.is_ge)
                mB = ps.tile([BLK, S], F32, tag="mB")
                nc.vector.tensor_scalar(mB[:], iota_t[:], sc_t[:, 1:2], None, Op.is_ge)
                msp = ps.tile([BLK, S], F32, tag="msp")
                nc.vector.scalar_tensor_tensor(
                    msp[:], iota_t[:], sc_t[:, 2:3], mB[:], op0=Op.is_lt, op1=Op.mult
                )
                inval = ps.tile([BLK, S], F32, tag="inval")
                nc.vector.tensor_tensor(inval[:], mA[:], msp[:], Op.max)

                # am = alpha - BIGC * invalid ; out = exp(am), sums over s
                am = ps.tile([BLK, S], F32, tag="am")
                nc.vector.scalar_tensor_tensor(
                    am[:], inval[:], -BIGC, alpha[:], op0=Op.mult, op1=Op.add
                )
                expv = pa.tile([BLK, S], F32, tag="expv")
                sums = ps.tile([BLK, 1], F32, tag="sums")
                nc.scalar.activation(
                    expv[:],
                    am[:],
                    mybir.ActivationFunctionType.Exp,
                    bias=0.0,
                    scale=1.0,
                    accum_out=sums[:],
                )
                recip = ps.tile([BLK, 1], F32, tag="recip")
                nc.vector.reciprocal(recip[:], sums[:])
                outt = pa.tile([BLK, S], F32, tag="outt")
                nc.vector.tensor_scalar(outt[:], expv[:], recip[:], None, Op.mult)
                nc.scalar.dma_start(out_ap[m0 : m0 + BLK, :], outt[:])

    nc.compile()
    _NC[rep] = nc
    return nc


def _host_prep(idx, lengths):
    idx = np.asarray(idx)
    lengths = np.asarray(lengths)
    sent = idx[:, 4].astype(np.int64)
    prefix = np.concatenate(
        [np.zeros(1, np.int64), np.cumsum(lengths.astype(np.int64))[:-1]]
    )
    mlen = lengths[sent].astype(np.float32)
    start = (idx[:, 2].astype(np.int64) - prefix[sent]).astype(np.float32)
    end = (idx[:, 3].astype(np.int64) - prefix[sent]).astype(np.float32)
    return np.stack([mlen, start, end], axis=1)  # [M, 3] f32


def kernel(queries, values, idx, lengths):
    from concourse.bass_utils import run_bass_kernel_spmd

    queries = np.ascontiguousarray(np.asarray(queries, dtype=np.float32))
    values = np.ascontiguousarray(np.asarray(values, dtype=np.float32))
    scal3 = _host_prep(idx, lengths)
    iota = np.ascontiguousarray(
        np.broadcast_to(np.arange(S, dtype=np.float32), (BLK, S))
    )

    nc = _build()
    in_maps = [
        {
            "queries": queries[c * ML : (c + 1) * ML],
            "values": values[c * ML : (c + 1) * ML],
            "scal3": scal3[c * ML : (c + 1) * ML],
            "iota": iota,
        }
        for c in range(NCORES)
    ]
    res = run_bass_kernel_spmd(nc, in_maps, core_ids=list(range(NCORES)))
    return np.concatenate([res.results[c]["out"] for c in range(NCORES)], axis=0)

